# revision 26
# baseline (speedup 1.0000x reference)
"""Graphformer layer (full multi-head attention) on 8 trn2 NeuronCores.

Sharding: one head per core (tensor parallel over the 8 heads).
Each core computes, for its head h:
    Q_h = x Wq_h^T, K_h = x Wk_h^T, V'_h = x (Wo_h Wv_h)^T   (4096, 64)
    S_h = Q_h K_h^T / 8;  P_h = softmax(S_h)                  (4096, 4096)
    y_core = P_h V'_h  (Wo folded into V on the host, fp32)   (4096, 64)
Full output = sum over cores + bo.   ~128.5us vs 598us fp32 baseline.

Design:
  - Q/K/S matmuls bf16; PV matmuls fp8e4 with DoubleRow perf mode
    (both key tiles of a pair contracted in one matmul at 0.5 cyc/col;
    v8 laid out [P, 2, JP, Da] so the pair-dim stride is 16-byte
    aligned as the ISA requires).
  - Q/K projections col-paired on the PE (wq -> out partitions 0:64,
    wk -> 64:128, running concurrently); V interleaved into the
    DMA-paced chunk loop.
  - S^T matmuls row-paired via tile_position (K=64: even key tile on
    PE rows 0:63, odd on 64:127), writing the two halves of one fused
    [128, 1024] PSUM tile.
  - exp of each fused tile in ONE op, alternating whole rounds between
    ScalarE (exact activation -> fp8) and VectorE (Schraudolph bit
    trick: fp8e4 bits = round(A*s + B) as a single tensor_scalar into
    a uint8 view; the approximation's multiplicative bias cancels in
    the softmax ratio).  Both engines measure >90% busy: this is the
    kernel's hard floor (only these two engines can read PSUM at rate).
  - softmax denominators ride as a ones column on V' (row 64 of the
    O^T accumulator); the augmented-identity output matmul transposes
    O^T back to row layout; normalization = DVE reciprocal + broadcast
    multiply reading PSUM directly.
  - input DMAs: host-packed weight layouts first, then x pieces
    alternating across both HWDGE rings (sync + scalar).
  - 2-round software pipeline skew so the PE never waits on exp.
"""

from contextlib import ExitStack

import numpy as np

import concourse.bass as bass
import concourse.bacc as bacc
import concourse.mybir as mybir
from concourse.tile import TileContext

N = 4096
C = 512  # input feature dim
D = 64  # head dim
Da = D + 1  # head dim + denominator column
HEADS = 8
P = 128
NT = N // P  # 32 key tiles
JP = NT // 2  # 16 key-tile pairs
CT = C // P  # 4 contraction tiles
F = 512  # query group width
G = N // F  # 8 query groups
F32 = mybir.dt.float32
BF16 = mybir.dt.bfloat16
FP8 = mybir.dt.float8e4
U8 = mybir.dt.uint8

# Schraudolph fast-exp constants for fp8e4 (e4m3) output bits:
#   e4m3_bits(exp(s/8)) ~= round(s * (log2(e)*8/8) + (7*8 - c))
# c = 8*(1.5 - 1/ln2) mean-centers the mantissa error (bias-free approx);
# +0.5 guards truncating float->uint8 converts.
EXP_A = 1.4426950408889634
EXP_B = 56.0 - 0.4584 + 0.5


def build_nc(use_dr=True, u8_exp=True, gp_ops=True, k2_rearr=True, dma_rearr=True):
    nc = bacc.Bacc()
    xT = nc.declare_dram_parameter("xT", [C, N], BF16, isOutput=False)
    wqk = nc.declare_dram_parameter("wqk", [P, CT * 2 * D], BF16, isOutput=False)
    wvT = nc.declare_dram_parameter("wvT", [P, CT * D], BF16, isOutput=False)
    woT = nc.declare_dram_parameter("woT", [Da, Da], BF16, isOutput=False)
    y = nc.declare_dram_parameter("y", [N, D], F32, isOutput=True)

    with TileContext(nc) as tc, ExitStack() as ctx:
        const = ctx.enter_context(tc.tile_pool(name="const", bufs=1))
        sb = ctx.enter_context(tc.tile_pool(name="sb", bufs=1))
        es_pool = ctx.enter_context(tc.tile_pool(name="es", bufs=6))
        ot_pool = ctx.enter_context(tc.tile_pool(name="ot", bufs=2))
        y_pool = ctx.enter_context(tc.tile_pool(name="yp", bufs=2))

        # preload the exp table set while DMAs run (first activation call
        # triggers the ~2.7us ACT_TABLE_LOAD).
        dum = const.tile([1, 2], F32, tag="dum")
        nc.vector.memset(dum[:, 0:1], 0.0)
        nc.scalar.activation(
            out=dum[:, 1:2],
            in_=dum[:, 0:1],
            func=mybir.ActivationFunctionType.Exp,
        )

        # ---- load inputs: weights FIRST, then x pieces alternating on the
        # two HWDGE rings (sync + scalar) for ~2x DMA bandwidth.
        wqk_sb = const.tile([P, CT, 2 * D], BF16, tag="wqk")
        wv_sb = const.tile([P, CT, D], BF16, tag="wv")
        nc.sync.dma_start(out=wqk_sb, in_=wqk[:, :])
        nc.scalar.dma_start(out=wv_sb, in_=wvT[:, :])
        wo_sb = const.tile([Da, Da], BF16, tag="wo")
        nc.sync.dma_start(out=wo_sb, in_=woT[:, :])

        xt = []
        for c in range(CT):
            xt.append(sb.tile([P, N], BF16, name=f"xt{c}", tag=f"xt{c}"))
        for piece in range(4):
            sl = slice(piece * 1024, (piece + 1) * 1024)
            for c in range(CT):
                eng = nc.sync if (piece * CT + c) % 2 == 0 else nc.scalar
                eng.dma_start(out=xt[c][:, sl], in_=xT[c * P : (c + 1) * P, sl])

        # ---- projections
        # q2: qT duplicated on both partition halves (rhs for both S row-tiles)
        # k2: kT pair-interleaved: pair j = key tile 2j on partitions 0:64,
        #     tile 2j+1 on partitions 64:128.
        # v8: fp8 V, pair-interleaved for DoubleRow: v8[:, j, i, :] = V tile
        #     2j+i (augmented with a ones column for the softmax denominator).
        q2 = sb.tile([P, N], BF16, tag="q2")
        k2 = sb.tile([P, JP, P], BF16, tag="k2")
        v8 = sb.tile([P, 2, JP, Da], FP8, tag="v8")
        with tc.tile_pool(name="psP", bufs=4, space="PSUM") as psP:
            for u in range(G):
                sl = slice(u * F, (u + 1) * F)
                pqk = psP.tile([P, F], F32, tag="pqk")
                for c in range(CT):
                    xs = xt[c][:, sl]
                    # col-paired: q -> out partitions 0:64, k -> 64:128
                    nc.tensor.matmul(
                        pqk[0:D, :],
                        wqk_sb[:, c, 0:D],
                        xs,
                        start=(c == 0),
                        stop=(c == CT - 1),
                    )
                    nc.tensor.matmul(
                        pqk[D : 2 * D, :],
                        wqk_sb[:, c, D : 2 * D],
                        xs,
                        start=(c == 0),
                        stop=(c == CT - 1),
                    )
                # qT: DVE copies lower half, GpSimd duplicates to upper half
                nc.vector.tensor_copy(out=q2[0:D, sl], in_=pqk[0:D, :])
                nc.vector.tensor_copy(out=q2[D:P, sl], in_=q2[0:D, sl])
                # kT pair-interleave: chunk u holds key tiles 4u..4u+3;
                # cols (t two c) with two=0 -> tiles 4u,4u+2 (lower halves),
                # two=1 -> tiles 4u+1,4u+3 (upper halves).
                if k2_rearr:
                    ksrc = pqk[D : 2 * D, :].rearrange(
                        "p (t two c) -> p t two c", two=2, c=P
                    )
                    nc.scalar.copy(out=k2[0:D, 2 * u : 2 * u + 2, :], in_=ksrc[:, :, 0, :])
                    nc.scalar.copy(out=k2[D:P, 2 * u : 2 * u + 2, :], in_=ksrc[:, :, 1, :])
                else:
                    for t in range(4):
                        j = 2 * u + t // 2
                        src_ = pqk[D : 2 * D, t * P : (t + 1) * P]
                        if t % 2 == 0:
                            nc.scalar.copy(out=k2[0:D, j, :], in_=src_)
                        else:
                            nc.scalar.copy(out=k2[D:P, j, :], in_=src_)
                # V (x-stationary) for the 4 key tiles this chunk just
                # streamed -- hides V under the DMA-paced qk loop
                for mt in range(4 * u, 4 * u + 4):
                    pv = psP.tile([P, D], F32, tag="pv")
                    for c in range(CT):
                        nc.tensor.matmul(
                            pv,
                            xt[c][:, mt * P : (mt + 1) * P],
                            wv_sb[:, c, :],
                            start=(c == 0),
                            stop=(c == CT - 1),
                        )
                    dst = v8[:, mt % 2, mt // 2, 0:D]
                    if mt % 2 == 0:
                        nc.vector.tensor_copy(out=dst, in_=pv)
                    else:
                        nc.scalar.copy(out=dst, in_=pv)
            nc.vector.memset(v8[:, :, :, D:Da], 1.0)

        # ---- attention + output projection, in query groups of F.
        # Each group's Y projection/normalization is deferred into the next
        # group's early rounds so the PE never stalls on the scalar ot-copy
        # at group boundaries.  py4 borrows the ss2 PSUM rotation; po is
        # double-buffered (6 + 2 = 8 banks).
        with (
            tc.tile_pool(name="psS", bufs=3, space="PSUM") as ps_s,
            tc.tile_pool(name="psO", bufs=1, space="PSUM") as ps_o,
            tc.tile_pool(name="psY", bufs=1, space="PSUM") as ps_y,
        ):

            def emit_y(gy, po_y):
                ot = ot_pool.tile([Da, F], BF16, tag="ot")
                nc.scalar.copy(out=ot, in_=po_y)
                py4 = ps_y.tile([P, F // P, Da], F32, tag="Y")
                for it in range(F // P):
                    nc.tensor.matmul(
                        py4[:, it, :],
                        ot[:, it * P : (it + 1) * P],
                        wo_sb,
                        start=True,
                        stop=True,
                    )
                rec = y_pool.tile([P, F // P, 1], F32, tag="rec")
                nc.vector.reciprocal(rec, py4[:, :, D:Da])
                ysb = y_pool.tile([P, F // P, D], F32, tag="ysb")
                nc.vector.tensor_mul(
                    ysb, py4[:, :, 0:D], rec.broadcast_to([P, F // P, D])
                )
                yv = y[gy * F : (gy + 1) * F, :].rearrange(
                    "(it p) d -> p it d", p=P
                )
                nc.sync.dma_start(out=yv, in_=ysb)

            for g in range(G):
                sl = slice(g * F, (g + 1) * F)
                po = ps_o.tile([Da, F], F32, tag="O")
                pend = []
                for j in range(JP + 2):
                    if j < JP:
                        # fused S^T pair tile: even key tile -> cols 0:F,
                        # odd -> cols F:2F (PE rows 0:63 / 64:127 concurrently)
                        ss2 = ps_s.tile([P, 2 * F], F32, tag="ss2")
                        nc.tensor.matmul(
                            ss2[:, 0:F], k2[0:D, j, :], q2[0:D, sl],
                            start=True, stop=True,
                        )
                        nc.tensor.matmul(
                            ss2[:, F : 2 * F], k2[D:P, j, :], q2[D:P, sl],
                            start=True, stop=True,
                        )
                        es2 = es_pool.tile([P, 2 * F], FP8, tag="es2")
                        if j % 2 == 0:
                            nc.scalar.activation(
                                out=es2,
                                in_=ss2,
                                func=mybir.ActivationFunctionType.Exp,
                                scale=0.125,
                            )
                        elif u8_exp:
                            nc.vector.tensor_scalar(
                                out=es2.bitcast(U8),
                                in0=ss2,
                                scalar1=EXP_A,
                                scalar2=EXP_B,
                                op0=mybir.AluOpType.mult,
                                op1=mybir.AluOpType.add,
                            )
                        else:
                            nc.vector.tensor_scalar(
                                out=es2,
                                in0=ss2,
                                scalar1=0.125,
                                scalar2=None,
                                op0=mybir.AluOpType.mult,
                            )
                        pend.append((es2, j))
                    if j >= 2:
                        e2, jj = pend.pop(0)
                        # DoubleRow: contracts both key tiles of the pair in
                        # one matmul (middle AP dim = 2 selects the sub-tile)
                        if use_dr:
                            nc.tensor.matmul(
                                po,
                                v8[:, :, jj, :],
                                e2.rearrange("p (two f) -> p two f", two=2),
                                start=(jj == 0),
                                stop=(jj == JP - 1),
                                perf_mode=mybir.MatmulPerfMode.DoubleRow,
                            )
                        else:
                            nc.tensor.matmul(
                                po, v8[:, 0, jj, :], e2[:, 0:F],
                                start=(jj == 0), stop=False,
                            )
                            nc.tensor.matmul(
                                po, v8[:, 1, jj, :], e2[:, F : 2 * F],
                                start=False, stop=(jj == JP - 1),
                            )
                emit_y(g, po)
    nc.compile()
    return nc


def make_in_maps(x, Wq, Wk, Wv, Wo):
    import ml_dtypes

    bf16 = ml_dtypes.bfloat16
    x = np.asarray(x, dtype=np.float32)
    Wq = np.asarray(Wq, dtype=np.float32)
    Wk = np.asarray(Wk, dtype=np.float32)
    Wv = np.asarray(Wv, dtype=np.float32)
    Wo = np.asarray(Wo, dtype=np.float32)
    xT = np.ascontiguousarray(x.T).astype(bf16)
    in_maps = []
    for h in range(HEADS):
        sl = slice(h * D, (h + 1) * D)
        woT = np.eye(Da, dtype=np.float32)
        wp = Wo[:, sl] @ Wv[sl]  # fold Wo into V (host fp32)
        wqk = np.concatenate(
            [np.ascontiguousarray(Wq[sl].T), np.ascontiguousarray(Wk[sl].T)], axis=1
        )  # [C, 2D]
        wqk = wqk.reshape(CT, P, 2 * D).transpose(1, 0, 2).reshape(P, CT * 2 * D)
        wpt = wp.T.reshape(CT, P, D).transpose(1, 0, 2).reshape(P, CT * D)
        in_maps.append(
            {
                "xT": xT,
                "wqk": np.ascontiguousarray(wqk).astype(bf16),
                "wvT": np.ascontiguousarray(wpt).astype(bf16),
                "woT": woT.astype(bf16),
            }
        )
    return in_maps


_CACHE = {}


def run_on_hw(x, Wq, Wk, Wv, Wo, bo, trace=False):
    from concourse.bass_utils import run_bass_kernel_spmd

    if "nc" not in _CACHE:
        _CACHE["nc"] = build_nc()
    nc = _CACHE["nc"]
    in_maps = make_in_maps(x, Wq, Wk, Wv, Wo)
    res = run_bass_kernel_spmd(nc, in_maps, list(range(HEADS)), trace=trace)
    out = np.zeros((N, D), np.float32)
    for r in res.results:
        out += r["y"]
    out += np.asarray(bo, dtype=np.float32)[None, :]
    return out, res


def kernel(x, Wq, Wk, Wv, Wo, bo):
    out, _ = run_on_hw(x, Wq, Wk, Wv, Wo, bo)
    return out
